# revision 9
# baseline (speedup 1.0000x reference)
"""Causal Grassmann Mixer — Trainium2 Bass kernel (8 NeuronCores, SPMD).

Sharding: data-parallel over B and sequence-parallel over L.
  core c -> batch b = c // 2, sequence half = c % 2 (2048 tokens each),
  plus a 32-token halo of h (the max offset) prepended on the host, so no
  cross-core communication is needed at all.

Device layout is feature-major everywhere: features on SBUF partitions,
tokens on the free dim.  All matmuls run in bf16 (fp32 PSUM accumulation)
except the gate's h-half, which runs in fp8 DoubleRow.

Math restructuring vs the reference (exact for the zero biases the
problem ships):
  z = h@red_w (R=16) in bf16, then ZI/ZJ gathered to 128 (120 + 8 zero
     pad) plucker lanes by a one-hot matmul, so the causal shift by d is
     just a column offset into the ZI/ZJ buffers.
  -> S = sum_d gelu(a_d); g = S @ (g2_w/6) + g2_b : one g2 matmul; the
     count division (6 for t>=32) folded into weights with an exact
     per-token correction on the first 512 tokens.
  -> GATE FUSION: g @ W2 = S @ W2' + g2_b@W2 with W2' = (g2_w/6) @ W2
     precomputed on the host -> the gate's g-half contracts over K=256
     (S, bf16) instead of K=1024 and g never needs an fp8 copy.
  -> norm: |p|^2 via one-hot PE reduce, rsqrt via Quake seed + 1 Newton
     step (fused scalar_tensor_tensor f32 ops), rinv broadcast to 128
     partitions on the GPSIMD engine (PE and scalar stay free).
  -> ramped segments (512 then 1536 tokens) so the first gate matmuls
     start ~40us earlier than a half/half split.
"""

import numpy as np
import ml_dtypes

B, L, D = 4, 4096, 1024
R = 16
PLU = 120
PLUP = 128          # padded plucker lanes (8 zero rows)
DG = 256
OFFSETS = (1, 2, 4, 8, 16, 32)
HALO = 32
IDX_I, IDX_J = np.triu_indices(R, k=1)

NCORES = 8
TOK = 2048          # own tokens per core
TB = TOK + HALO     # token buffer incl. halo
T = 512             # PSUM bank tile (fp32)
SEGS = ((0, 1), (1, 3))   # (first tile, n tiles): ramped pipeline
KD = D // 128       # 8 k-chunks of the model dim
WSC = 64.0          # fp8 gate-weight scale (descaled in the sigmoid)

BF16 = ml_dtypes.bfloat16

_CACHE = {}

# rb broadcast path: "gpsimd" (partition_broadcast) or "pe" (selector matmul)
BCAST = "pe"


def _build_program():
    import concourse.bass as bass
    import concourse.mybir as mybir
    import concourse.tile as tile
    from concourse import bacc

    f32 = mybir.dt.float32
    bf16 = mybir.dt.bfloat16
    f8 = mybir.dt.float8e4
    i32 = mybir.dt.int32
    AF = mybir.ActivationFunctionType
    ALU = mybir.AluOpType
    DR = mybir.MatmulPerfMode.DoubleRow

    nc = bacc.Bacc(
        "TRN2",
        target_bir_lowering=False,
        debug=False,
        enable_asserts=False,
        num_devices=NCORES,
    )

    NSMAX = 6 * max(n for _, n in SEGS)

    # ---- DRAM I/O ----
    h_t = nc.dram_tensor("h_t", [D, TB], bf16, kind="ExternalInput").ap()
    h8_t = nc.dram_tensor("h8_t", [D, TB], f8, kind="ExternalInput").ap()
    rw = nc.dram_tensor("rw", [D, 32], bf16, kind="ExternalInput").ap()      # 16 pad cols
    rb16 = nc.dram_tensor("rb16", [128, 1], f32, kind="ExternalInput").ap()  # bias on strip 0
    selij = nc.dram_tensor("selij", [128, 2 * PLUP], bf16, kind="ExternalInput").ap()
    g1w = nc.dram_tensor("g1w", [PLUP, DG], bf16, kind="ExternalInput").ap()
    g1b = nc.dram_tensor("g1b", [128, 2], f32, kind="ExternalInput").ap()
    g2w = nc.dram_tensor("g2w", [DG, D], bf16, kind="ExternalInput").ap()   # g2_w/6
    g2b = nc.dram_tensor("g2b", [128, KD], f32, kind="ExternalInput").ap()  # g2_b
    w2p = nc.dram_tensor("w2p", [DG, D], bf16, kind="ExternalInput").ap()   # (g2w/6)@W2*WSC
    gw1 = nc.dram_tensor("gw1", [D, D], f8, kind="ExternalInput").ap()      # W1*WSC
    gtb = nc.dram_tensor("gtb", [128, KD], f32, kind="ExternalInput").ap()  # gate_b+g2b@W2
    corr = nc.dram_tensor("corr", [1, T], bf16, kind="ExternalInput").ap()
    rsel_d = nc.dram_tensor("rsel", [NSMAX, NSMAX * PLUP], bf16, kind="ExternalInput").ap()
    out_t = nc.dram_tensor("out_t", [D, TOK], bf16, kind="ExternalOutput").ap()

    with tile.TileContext(nc) as tc:
        from contextlib import ExitStack

        ctx = ExitStack()
        with ctx:
            singles = ctx.enter_context(tc.tile_pool(name="singles", bufs=1))
            work = ctx.enter_context(tc.tile_pool(name="work", bufs=3))
            rwk = ctx.enter_context(tc.tile_pool(name="rwk", bufs=4))
            rbp = ctx.enter_context(tc.tile_pool(name="rbp", bufs=4))
            psul = ctx.enter_context(tc.tile_pool(name="psul", bufs=4, space="PSUM"))
            psg = ctx.enter_context(tc.tile_pool(name="psg", bufs=2, space="PSUM"))
            pss = ctx.enter_context(tc.tile_pool(name="pss", bufs=2, space="PSUM"))

            # ---- resident SBUF tensors (weights first: small, unblock z) ----
            rw_sb = singles.tile([128, KD, 32], bf16)
            nc.sync.dma_start(out=rw_sb, in_=rw.rearrange("(c p) m -> p c m", p=128))
            rb_sb = singles.tile([128, 1], f32)
            nc.sync.dma_start(out=rb_sb, in_=rb16)
            sel_sb = singles.tile([128, 2 * PLUP], bf16)
            nc.sync.dma_start(out=sel_sb, in_=selij)
            # h chunks 0-2 next: the z phase needs them before anything else
            h_sb = singles.tile([128, KD, TB], bf16)
            h_r = h_t.rearrange("(c p) t -> p c t", p=128)
            zchunks = [(c * T, min(T, TB - c * T)) for c in range((TB + T - 1) // T)]
            for (c0, csz) in zchunks[:3]:
                for k in range(KD):
                    nc.sync.dma_start(
                        out=h_sb[:, k, c0:c0 + csz], in_=h_r[:, k, c0:c0 + csz]
                    )
            g1w_sb = singles.tile([PLUP, DG], bf16)
            nc.sync.dma_start(out=g1w_sb, in_=g1w)
            g1b_sb = singles.tile([128, 2], f32)
            nc.sync.dma_start(out=g1b_sb, in_=g1b)
            rsel = None
            if BCAST == "pe":
                rsel = singles.tile([NSMAX, NSMAX, PLUP], bf16)
                nc.sync.dma_start(
                    out=rsel, in_=rsel_d.rearrange("k (d m) -> k d m", m=PLUP)
                )
            corr_sb = singles.tile([1, T], bf16)
            nc.sync.dma_start(out=corr_sb, in_=corr)
            for (c0, csz) in zchunks[3:]:
                for k in range(KD):
                    nc.sync.dma_start(
                        out=h_sb[:, k, c0:c0 + csz], in_=h_r[:, k, c0:c0 + csz]
                    )
            g2w_sb = singles.tile([128, 2, D], bf16)
            nc.sync.dma_start(out=g2w_sb, in_=g2w.rearrange("(c p) m -> p c m", p=128))
            g2b_sb = singles.tile([128, KD], f32)
            nc.sync.dma_start(out=g2b_sb, in_=g2b)
            w2p_sb = singles.tile([128, 2, D], bf16)
            nc.sync.dma_start(out=w2p_sb, in_=w2p.rearrange("(c p) m -> p c m", p=128))
            gw1_sb = singles.tile([128, KD, D], f8)
            nc.sync.dma_start(out=gw1_sb, in_=gw1.rearrange("(c p) m -> p c m", p=128))
            gtb_sb = singles.tile([128, KD], f32)
            nc.sync.dma_start(out=gtb_sb, in_=gtb)

            ones_m = singles.tile([1, 128], bf16)
            nc.vector.memset(ones_m, 1.0)
            onehot = singles.tile([PLUP, NSMAX, NSMAX], bf16)
            nc.vector.memset(onehot, 0.0)
            for dcol in range(NSMAX):
                nc.vector.memset(onehot[:, dcol, dcol:dcol + 1], 1.0)
            magic = singles.tile([NSMAX, T], i32)
            nc.vector.memset(magic, 0x5F375A86)  # Quake rsqrt seed

            # h in fp8 (gate rhs)
            h8_sb = singles.tile([128, KD, TB], f8)
            h8_r = h8_t.rearrange("(c p) t -> p c t", p=128)
            for k in range(KD):
                nc.sync.dma_start(out=h8_sb[:, k, :], in_=h8_r[:, k, :])

            # corr broadcast to 128 partitions (once)
            corr_ps = pss.tile([128, T], f32, name="corrps", tag="ps")
            nc.tensor.matmul(corr_ps, lhsT=ones_m, rhs=corr_sb, start=True, stop=True)
            corr128 = singles.tile([128, T], bf16)
            nc.vector.tensor_copy(out=corr128, in_=corr_ps)

            zi_sb = singles.tile([PLUP, TB], bf16)
            zj_sb = singles.tile([PLUP, TB], bf16)
            pp_pool = ctx.enter_context(tc.tile_pool(name="pp", bufs=1))
            sq_pool = ctx.enter_context(tc.tile_pool(name="sqp", bufs=1))
            s_pool = ctx.enter_context(tc.tile_pool(name="spool", bufs=1))
            gfm_pool = ctx.enter_context(tc.tile_pool(name="gfmp", bufs=1))
            st = {}

            # ---- phase Z ----
            def zphase(chunks):
                for (c0, csz) in chunks:
                    zp = pss.tile([128, T], f32, tag="ps")
                    for j in range(4):
                        for k2 in range(2):
                            nc.tensor.matmul(
                                zp[32 * j:32 * j + 32, :csz],
                                lhsT=rw_sb[:, 2 * j + k2, :],
                                rhs=h_sb[:, 2 * j + k2, c0:c0 + csz],
                                start=(k2 == 0),
                                stop=(k2 == 1),
                                tile_position=(0, 32 * j),
                            )
                    z16 = work.tile([128, T], bf16, tag="z16", bufs=2)
                    nc.vector.tensor_scalar_add(z16[:, :csz], zp[:, :csz], rb_sb)
                    for g, z_sb in ((0, zi_sb), (1, zj_sb)):
                        gp = pss.tile([PLUP, T], f32, tag="ps")
                        nc.tensor.matmul(
                            gp[:, :csz],
                            lhsT=sel_sb[:, g * PLUP:(g + 1) * PLUP],
                            rhs=z16[:, :csz],
                            start=True,
                            stop=True,
                        )
                        nc.vector.tensor_copy(out=z_sb[:, c0:c0 + csz], in_=gp[:, :csz])

            out_r = out_t.rearrange("(c p) t -> p c t", p=128)

            def p1a(si):
                """DVE: plucker p per tile; p^2 too (kept on DVE)."""
                t0, nt = SEGS[si]
                GL = nt * T
                g0 = HALO + t0 * T
                pp = pp_pool.tile([PLUP, 6, GL], bf16, name=f"pp{si}", tag="pp")
                sq6 = sq_pool.tile([PLUP, 6, GL], bf16, name=f"sq{si}", tag="sq")
                st[si] = {"pp": pp, "sq6": sq6}
                for di, delta in enumerate(OFFSETS):
                    for i in range(nt):
                        past = slice(g0 + i * T - delta, g0 + (i + 1) * T - delta)
                        cur = slice(g0 + i * T, g0 + (i + 1) * T)
                        sl = slice(i * T, (i + 1) * T)
                        m1 = work.tile([PLUP, T], bf16, tag="m1")
                        nc.vector.tensor_mul(m1, zi_sb[:, past], zj_sb[:, cur])
                        m2 = work.tile([PLUP, T], bf16, tag="m2")
                        nc.vector.tensor_mul(m2, zj_sb[:, past], zi_sb[:, cur])
                        nc.vector.tensor_sub(pp[:, di, sl], m1, m2)
                    # p^2 on the (otherwise idle) scalar engine
                    nc.scalar.activation(sq6[:, di, :], pp[:, di, :], AF.Square)

            def norm(si):
                """PE reduce |p|^2 -> Quake rsqrt -> broadcast -> scale."""
                t0, nt = SEGS[si]
                pp, sq6 = st[si]["pp"], st[si]["sq6"]
                NS = 6 * nt
                ns = pss.tile([NSMAX, T], f32, tag="ps", name=f"ns{si}")
                first = True
                for di in range(6):
                    for i in range(nt):
                        nc.tensor.matmul(
                            ns[:NS, :],
                            lhsT=onehot[:, 6 * i + di, :NS],
                            rhs=sq6[:, di, i * T:(i + 1) * T],
                            start=first,
                            stop=(di == 5 and i == nt - 1),
                        )
                        first = False
                # rinv = rsqrt(ns + EPS^2): Quake seed + 1 Newton step
                nsf = rwk.tile([NSMAX, T], f32, tag="rs")
                nc.vector.tensor_scalar_add(nsf[:NS], ns[:NS], 1e-12)
                sh = rwk.tile([NSMAX, T], i32, tag="rs")
                nc.vector.tensor_scalar(
                    sh[:NS], nsf[:NS].bitcast(i32), 1, None,
                    op0=ALU.arith_shift_right,
                )
                y0 = rwk.tile([NSMAX, T], f32, tag="rs")
                nc.vector.tensor_sub(y0[:NS].bitcast(i32), magic[:NS], sh[:NS])
                t1 = rwk.tile([NSMAX, T], f32, tag="rs")
                nc.vector.tensor_mul(t1[:NS], y0[:NS], y0[:NS])
                # t1 = -0.5*y0^2*nsf ; rinv = (t1 + 1.5)*y0
                nc.vector.scalar_tensor_tensor(
                    t1[:NS], t1[:NS], -0.5, nsf[:NS], op0=ALU.mult, op1=ALU.mult,
                )
                rinv = rwk.tile([NSMAX, T], bf16, tag="ri", bufs=2)
                nc.vector.scalar_tensor_tensor(
                    rinv[:NS], t1[:NS], 1.5, y0[:NS], op0=ALU.add, op1=ALU.mult,
                )
                for di in range(6):
                    for i in range(nt):
                        sl = slice(i * T, (i + 1) * T)
                        if BCAST == "gpsimd":
                            rb2 = rbp.tile([128, T], bf16, tag="rb")
                            nc.gpsimd.partition_broadcast(
                                rb2, rinv[6 * i + di:6 * i + di + 1, :]
                            )
                            nc.vector.tensor_mul(
                                pp[:, di, sl], pp[:, di, sl], rb2
                            )
                        else:
                            rb_ps = pss.tile([PLUP, T], f32, tag="ps")
                            nc.tensor.matmul(
                                rb_ps, lhsT=rsel[:NS, 6 * i + di, :],
                                rhs=rinv[:NS, :], start=True, stop=True,
                            )
                            nc.vector.tensor_mul(
                                pp[:, di, sl], pp[:, di, sl], rb_ps
                            )

            def p2(si):
                """a_d = p@g1_w; S = sum_d gelu(a_d + g1_b)."""
                t0, nt = SEGS[si]
                GL = nt * T
                pp = st[si]["pp"]
                s_sb = s_pool.tile([128, 2, GL], bf16, name=f"s{si}", tag="s")
                st[si]["s"] = s_sb
                for di in range(6):
                    for i in range(nt):
                        sl = slice(i * T, (i + 1) * T)
                        for m in range(2):
                            ap_ps = psg.tile([128, T], f32, tag="pg")
                            nc.tensor.matmul(
                                ap_ps,
                                lhsT=g1w_sb[:, m * 128:(m + 1) * 128],
                                rhs=pp[:, di, sl],
                                start=True,
                                stop=True,
                            )
                            if di == 0:
                                nc.scalar.activation(
                                    s_sb[:, m, sl], ap_ps, AF.Gelu,
                                    bias=g1b_sb[:, m:m + 1],
                                )
                            else:
                                gt = work.tile([128, T], bf16, tag="gt")
                                nc.scalar.activation(
                                    gt, ap_ps, AF.Gelu, bias=g1b_sb[:, m:m + 1]
                                )
                                nc.vector.tensor_add(
                                    s_sb[:, m, sl], s_sb[:, m, sl], gt
                                )
                if si == 0:
                    # first-tile count correction (corr==1 for t>=32)
                    for m in range(2):
                        nc.vector.tensor_mul(
                            s_sb[:, m, 0:T], s_sb[:, m, 0:T], corr128
                        )

            def g2part(si):
                """gfm = S @ (g2_w/6) + g2_b (bf16, blend input)."""
                t0, nt = SEGS[si]
                GL = nt * T
                s_sb = st[si]["s"]
                gfm = gfm_pool.tile([128, KD, GL], bf16, name=f"gfm{si}", tag="gfm")
                st[si]["gfm"] = gfm
                for i in range(nt):
                    sl = slice(i * T, (i + 1) * T)
                    for m8 in range(KD):
                        gp = psg.tile([128, T], f32, tag="pg")
                        for k2 in range(2):
                            nc.tensor.matmul(
                                gp,
                                lhsT=g2w_sb[:, k2, m8 * 128:(m8 + 1) * 128],
                                rhs=s_sb[:, k2, sl],
                                start=(k2 == 0),
                                stop=(k2 == 1),
                            )
                        if m8 % 2 == 0:
                            nc.scalar.activation(
                                gfm[:, m8, sl], gp, AF.Identity,
                                bias=g2b_sb[:, m8:m8 + 1],
                            )
                        else:
                            nc.vector.tensor_scalar_add(
                                gfm[:, m8, sl], gp, g2b_sb[:, m8:m8 + 1]
                            )

            def gate(si):
                """logits = h8@(W1*WSC) [fp8 DR] + S@(W2'*WSC) [bf16] in one
                PSUM; alpha = sigmoid(l/WSC + gtb); blend; store."""
                t0, nt = SEGS[si]
                s_sb = st[si]["s"]
                gfm = st[si]["gfm"]
                for i in range(nt):
                    sl = slice(i * T, (i + 1) * T)
                    ti = t0 + i
                    cur = slice(HALO + ti * T, HALO + (ti + 1) * T)
                    for mb in range(2):
                        lps = {}
                        for m8 in range(4 * mb, 4 * mb + 4):
                            lp = psul.tile([128, T], f32, tag="lp")
                            lps[m8] = lp
                            ms = slice(m8 * 128, (m8 + 1) * 128)
                            for kp in range(KD // 2):
                                nc.tensor.matmul(
                                    lp,
                                    lhsT=gw1_sb[:, 2 * kp:2 * kp + 2, ms],
                                    rhs=h8_sb[:, 2 * kp:2 * kp + 2, cur],
                                    start=(kp == 0),
                                    stop=False,
                                    perf_mode=DR,
                                )
                        for m8 in range(4 * mb, 4 * mb + 4):
                            lp = lps[m8]
                            ms = slice(m8 * 128, (m8 + 1) * 128)
                            for k2 in range(2):
                                nc.tensor.matmul(
                                    lp,
                                    lhsT=w2p_sb[:, k2, ms],
                                    rhs=s_sb[:, k2, sl],
                                    start=False,
                                    stop=(k2 == 1),
                                )
                            alpha = work.tile([128, T], bf16, tag="al")
                            nc.scalar.activation(
                                alpha, lp, AF.Sigmoid, bias=gtb_sb[:, m8:m8 + 1],
                                scale=1.0 / WSC,
                            )
                            dd = work.tile([128, T], bf16, tag="dd")
                            if m8 % 4 == 0:
                                nc.gpsimd.tensor_sub(
                                    dd, h_sb[:, m8, cur], gfm[:, m8, sl]
                                )
                            else:
                                nc.vector.tensor_sub(
                                    dd, h_sb[:, m8, cur], gfm[:, m8, sl]
                                )
                            mm = work.tile([128, T], bf16, tag="mm")
                            nc.vector.tensor_mul(mm, alpha, dd)
                            oo = work.tile([128, T], bf16, tag="oo")
                            nc.vector.tensor_add(oo, gfm[:, m8, sl], mm)
                            nc.sync.dma_start(
                                out=out_r[:, m8, ti * T:(ti + 1) * T], in_=oo
                            )

            # ---- ramped software pipeline ----
            zphase(zchunks[:2])
            p1a(0)
            zphase(zchunks[2:3])
            norm(0)
            p2(0)
            g2part(0)
            zphase(zchunks[3:])
            p1a(1)
            gate(0)
            norm(1)
            p2(1)
            g2part(1)
            gate(1)

    nc.compile()
    return nc


def _get_program():
    if "nc" not in _CACHE:
        _CACHE["nc"] = _build_program()
    return _CACHE["nc"]


def make_in_maps(h, red_w, red_b, g1_w, g1_b, g2_w, g2_b, gate_w, gate_b):
    """Host-side sharding + layout prep. Returns list of 8 input dicts."""
    h = np.asarray(h, np.float32)
    red_w = np.asarray(red_w, np.float32)
    red_b = np.asarray(red_b, np.float32)
    g1_w = np.asarray(g1_w, np.float32)
    g1_b = np.asarray(g1_b, np.float32)
    g2_w = np.asarray(g2_w, np.float32)
    g2_b = np.asarray(g2_b, np.float32)
    gate_w = np.asarray(gate_w, np.float32)
    gate_b = np.asarray(gate_b, np.float32)

    from concourse import mybir as _mb
    F8 = _mb.dt.np(_mb.dt.float8e4)

    W1 = gate_w[:D]
    W2 = gate_w[D:]

    rwp = np.zeros((D, 32), np.float32)
    rwp[:, :R] = red_w
    rw = np.ascontiguousarray(rwp.astype(BF16))
    rb4 = np.zeros((128, 1), np.float32)
    rb4[:R, 0] = red_b
    rb16 = np.ascontiguousarray(rb4)
    selij = np.zeros((128, 2 * PLUP), np.float32)
    for j in range(4):
        for k in range(PLU):
            selij[32 * j + IDX_I[k], k] = 1.0
            selij[32 * j + IDX_J[k], PLUP + k] = 1.0
    selij = np.ascontiguousarray(selij.astype(BF16))
    g1w = np.zeros((PLUP, DG), np.float32)
    g1w[:PLU] = g1_w
    g1w = np.ascontiguousarray(g1w.astype(BF16))
    g1b = np.ascontiguousarray(g1_b.reshape(2, 128).T.astype(np.float32))
    g2wd = np.ascontiguousarray((g2_w / 6.0).astype(BF16))
    g2bd = np.ascontiguousarray(g2_b.reshape(KD, 128).T.astype(np.float32))
    w2p = np.ascontiguousarray((((g2_w / 6.0) @ W2) * WSC).astype(BF16))
    gw1 = np.ascontiguousarray((W1 * WSC).astype(F8))
    gtbd = np.ascontiguousarray(
        (gate_b + g2_b @ W2).reshape(KD, 128).T.astype(np.float32)
    )

    t = np.arange(T)
    count = np.zeros(T, np.float32)
    for d in OFFSETS:
        count += (t >= d)
    corr0 = np.where(count > 0, 6.0 / np.maximum(count, 1.0), 0.0).astype(BF16)
    corr0 = corr0.reshape(1, T)
    corr1 = np.ones((1, T), BF16)

    NSMAX = 6 * max(n for _, n in SEGS)
    rsel = np.zeros((NSMAX, NSMAX, PLUP), np.float32)
    for dd in range(NSMAX):
        rsel[dd, dd, :] = 1.0
    rsel = np.ascontiguousarray(rsel.reshape(NSMAX, NSMAX * PLUP).astype(BF16))

    in_maps = []
    for c in range(NCORES):
        b, half = c // 2, c % 2
        if half == 0:
            pad = np.zeros((HALO, D), np.float32)
        else:
            pad = h[b, half * TOK - HALO: half * TOK]
        hs = np.concatenate([pad, h[b, half * TOK:(half + 1) * TOK]], axis=0)
        h_tt = np.ascontiguousarray(hs.T.astype(BF16))  # (D, TB)
        h8_t = np.ascontiguousarray(hs.T.astype(F8))
        in_maps.append({
            "h_t": h_tt,
            "h8_t": h8_t,
            "rw": rw,
            "rb16": rb16,
            "selij": selij,
            "g1w": g1w,
            "g1b": g1b,
            "g2w": g2wd,
            "g2b": g2bd,
            "w2p": w2p,
            "gw1": gw1,
            "gtb": gtbd,
            "corr": corr0 if half == 0 else corr1,
            "rsel": rsel,
        })
    return in_maps


def assemble_output(results):
    out = np.empty((B, L, D), np.float32)
    for c in range(NCORES):
        b, half = c // 2, c % 2
        ot = np.asarray(results[c]["out_t"]).astype(np.float32)  # (D, TOK)
        out[b, half * TOK:(half + 1) * TOK, :] = ot.T
    return out


def kernel(**inputs):
    from concourse.bass_utils import run_bass_kernel_spmd

    nc = _get_program()
    in_maps = make_in_maps(**inputs)
    res = run_bass_kernel_spmd(nc, in_maps, core_ids=list(range(NCORES)))
    return assemble_output(res.results)


# revision 10
# speedup vs baseline: 1.1208x; 1.1208x over previous
"""Causal Grassmann Mixer — Trainium2 Bass kernel (8 NeuronCores, SPMD).

Sharding: data-parallel over B and sequence-parallel over L.
  core c -> batch b = c // 2, sequence half = c % 2 (2048 tokens each),
  plus a 32-token halo of h (the max offset) prepended on the host, so no
  cross-core communication is needed at all.

Device layout is feature-major everywhere: features on SBUF partitions,
tokens on the free dim.  All matmuls run in bf16 (fp32 PSUM accumulation)
except the gate's h-half, which runs in fp8 DoubleRow.

Math restructuring vs the reference (exact for the zero biases the
problem ships):
  z = h@red_w (R=16) in bf16, then ZI/ZJ gathered to 128 (120 + 8 zero
     pad) plucker lanes by a one-hot matmul, so the causal shift by d is
     just a column offset into the ZI/ZJ buffers.
  -> S = sum_d gelu(a_d); g = S @ (g2_w/6) + g2_b : one g2 matmul; the
     count division (6 for t>=32) folded into weights with an exact
     per-token correction on the first 512 tokens.
  -> GATE FUSION: g @ W2 = S @ W2' + g2_b@W2 with W2' = (g2_w/6) @ W2
     precomputed on the host -> the gate's g-half contracts over K=256
     (S, bf16) instead of K=1024 and g never needs an fp8 copy.
  -> norm: |p|^2 via one-hot PE reduce, rsqrt via Quake seed + 1 Newton
     step (fused scalar_tensor_tensor f32 ops), rinv broadcast to 128
     partitions on the GPSIMD engine (PE and scalar stay free).
  -> ramped segments (512 then 1536 tokens) so the first gate matmuls
     start ~40us earlier than a half/half split.
"""

import numpy as np
import ml_dtypes

B, L, D = 4, 4096, 1024
R = 16
PLU = 120
PLUP = 128          # padded plucker lanes (8 zero rows)
DG = 256
OFFSETS = (1, 2, 4, 8, 16, 32)
HALO = 32
IDX_I, IDX_J = np.triu_indices(R, k=1)

NCORES = 8
TOK = 2048          # own tokens per core
TB = TOK + HALO     # token buffer incl. halo
T = 512             # PSUM bank tile (fp32)
SEGS = ((0, 1), (1, 3))   # (first tile, n tiles): ramped pipeline
KD = D // 128       # 8 k-chunks of the model dim
WSC = 64.0          # fp8 gate-weight scale (descaled in the sigmoid)

BF16 = ml_dtypes.bfloat16

_CACHE = {}

# rb broadcast path: "gpsimd" (partition_broadcast) or "pe" (selector matmul)
BCAST = "pe"


def _build_program():
    import concourse.bass as bass
    import concourse.mybir as mybir
    import concourse.tile as tile
    from concourse import bacc

    f32 = mybir.dt.float32
    bf16 = mybir.dt.bfloat16
    f8 = mybir.dt.float8e4
    i32 = mybir.dt.int32
    AF = mybir.ActivationFunctionType
    ALU = mybir.AluOpType
    DR = mybir.MatmulPerfMode.DoubleRow

    nc = bacc.Bacc(
        "TRN2",
        target_bir_lowering=False,
        debug=False,
        enable_asserts=False,
        num_devices=NCORES,
    )

    NSMAX = 6 * max(n for _, n in SEGS)

    # ---- DRAM I/O ----
    h_t = nc.dram_tensor("h_t", [D, TB], bf16, kind="ExternalInput").ap()
    h8_t = nc.dram_tensor("h8_t", [D, TB], f8, kind="ExternalInput").ap()
    rw = nc.dram_tensor("rw", [D, 32], bf16, kind="ExternalInput").ap()      # 16 pad cols
    rb16 = nc.dram_tensor("rb16", [128, 1], f32, kind="ExternalInput").ap()  # bias on strip 0
    selij = nc.dram_tensor("selij", [128, 2 * PLUP], bf16, kind="ExternalInput").ap()
    g1w = nc.dram_tensor("g1w", [PLUP, DG], bf16, kind="ExternalInput").ap()
    g1b = nc.dram_tensor("g1b", [128, 2], f32, kind="ExternalInput").ap()
    g2w = nc.dram_tensor("g2w", [DG, D], bf16, kind="ExternalInput").ap()   # g2_w/6
    g2b = nc.dram_tensor("g2b", [128, KD], f32, kind="ExternalInput").ap()  # g2_b
    w2p = nc.dram_tensor("w2p", [DG, D], bf16, kind="ExternalInput").ap()   # (g2w/6)@W2*WSC
    gw1 = nc.dram_tensor("gw1", [D, D], f8, kind="ExternalInput").ap()      # W1*WSC
    gtb = nc.dram_tensor("gtb", [128, KD], f32, kind="ExternalInput").ap()  # gate_b+g2b@W2
    corr = nc.dram_tensor("corr", [1, T], bf16, kind="ExternalInput").ap()
    rsel_d = nc.dram_tensor("rsel", [NSMAX, NSMAX * PLUP], bf16, kind="ExternalInput").ap()
    out_t = nc.dram_tensor("out_t", [D, TOK], bf16, kind="ExternalOutput").ap()

    with tile.TileContext(nc) as tc:
        from contextlib import ExitStack

        ctx = ExitStack()
        with ctx:
            singles = ctx.enter_context(tc.tile_pool(name="singles", bufs=1))
            work = ctx.enter_context(tc.tile_pool(name="work", bufs=3))
            rwk = ctx.enter_context(tc.tile_pool(name="rwk", bufs=4))
            rbp = ctx.enter_context(tc.tile_pool(name="rbp", bufs=4))
            psul = ctx.enter_context(tc.tile_pool(name="psul", bufs=4, space="PSUM"))
            psg = ctx.enter_context(tc.tile_pool(name="psg", bufs=2, space="PSUM"))
            pss = ctx.enter_context(tc.tile_pool(name="pss", bufs=2, space="PSUM"))

            # ---- resident SBUF tensors (weights first: small, unblock z) ----
            rw_sb = singles.tile([128, KD, 32], bf16)
            nc.sync.dma_start(out=rw_sb, in_=rw.rearrange("(c p) m -> p c m", p=128))
            rb_sb = singles.tile([128, 1], f32)
            nc.sync.dma_start(out=rb_sb, in_=rb16)
            sel_sb = singles.tile([128, 2 * PLUP], bf16)
            nc.sync.dma_start(out=sel_sb, in_=selij)
            corr_sb = singles.tile([1, T], bf16)
            nc.sync.dma_start(out=corr_sb, in_=corr)
            # h chunks 0-2 next: the z phase needs them before anything else
            h_sb = singles.tile([128, KD, TB], bf16)
            h_r = h_t.rearrange("(c p) t -> p c t", p=128)
            zchunks = [(c * T, min(T, TB - c * T)) for c in range((TB + T - 1) // T)]
            for (c0, csz) in zchunks[:3]:
                for k in range(KD):
                    nc.sync.dma_start(
                        out=h_sb[:, k, c0:c0 + csz], in_=h_r[:, k, c0:c0 + csz]
                    )
            g1w_sb = singles.tile([PLUP, DG], bf16)
            nc.sync.dma_start(out=g1w_sb, in_=g1w)
            g1b_sb = singles.tile([128, 2], f32)
            nc.sync.dma_start(out=g1b_sb, in_=g1b)
            rsel = None
            if BCAST == "pe":
                rsel = singles.tile([NSMAX, NSMAX, PLUP], bf16)
                nc.sync.dma_start(
                    out=rsel, in_=rsel_d.rearrange("k (d m) -> k d m", m=PLUP)
                )
            for (c0, csz) in zchunks[3:]:
                for k in range(KD):
                    nc.sync.dma_start(
                        out=h_sb[:, k, c0:c0 + csz], in_=h_r[:, k, c0:c0 + csz]
                    )
            g2w_sb = singles.tile([128, 2, D], bf16)
            nc.sync.dma_start(out=g2w_sb, in_=g2w.rearrange("(c p) m -> p c m", p=128))
            g2b_sb = singles.tile([128, KD], f32)
            nc.sync.dma_start(out=g2b_sb, in_=g2b)
            w2p_sb = singles.tile([128, 2, D], bf16)
            nc.sync.dma_start(out=w2p_sb, in_=w2p.rearrange("(c p) m -> p c m", p=128))
            gw1_sb = singles.tile([128, KD, D], f8)
            nc.sync.dma_start(out=gw1_sb, in_=gw1.rearrange("(c p) m -> p c m", p=128))
            gtb_sb = singles.tile([128, KD], f32)
            nc.sync.dma_start(out=gtb_sb, in_=gtb)

            ones_m = singles.tile([1, 128], bf16)
            nc.vector.memset(ones_m, 1.0)
            onehot = singles.tile([PLUP, NSMAX, NSMAX], bf16)
            nc.vector.memset(onehot, 0.0)
            for dcol in range(NSMAX):
                nc.vector.memset(onehot[:, dcol, dcol:dcol + 1], 1.0)
            magic = singles.tile([NSMAX, T], i32)
            nc.vector.memset(magic, 0x5F375A86)  # Quake rsqrt seed

            # h in fp8 (gate rhs)
            h8_sb = singles.tile([128, KD, TB], f8)
            h8_r = h8_t.rearrange("(c p) t -> p c t", p=128)
            for k in range(KD):
                nc.sync.dma_start(out=h8_sb[:, k, :], in_=h8_r[:, k, :])

            corr128 = singles.tile([128, T], bf16)

            def corr_bcast():
                corr_ps = pss.tile([128, T], f32, name="corrps", tag="ps")
                nc.tensor.matmul(
                    corr_ps, lhsT=ones_m, rhs=corr_sb, start=True, stop=True
                )
                nc.vector.tensor_copy(out=corr128, in_=corr_ps)

            zi_sb = singles.tile([PLUP, TB], bf16)
            zj_sb = singles.tile([PLUP, TB], bf16)
            pp_pool = ctx.enter_context(tc.tile_pool(name="pp", bufs=1))
            sq_pool = ctx.enter_context(tc.tile_pool(name="sqp", bufs=1))
            s_pool = ctx.enter_context(tc.tile_pool(name="spool", bufs=1))
            gfm_pool = ctx.enter_context(tc.tile_pool(name="gfmp", bufs=1))
            st = {}

            # ---- phase Z ----
            def zphase(chunks):
                for (c0, csz) in chunks:
                    zp = pss.tile([128, T], f32, tag="ps")
                    for j in range(4):
                        for k2 in range(2):
                            nc.tensor.matmul(
                                zp[32 * j:32 * j + 32, :csz],
                                lhsT=rw_sb[:, 2 * j + k2, :],
                                rhs=h_sb[:, 2 * j + k2, c0:c0 + csz],
                                start=(k2 == 0),
                                stop=(k2 == 1),
                                tile_position=(0, 32 * j),
                            )
                    z16 = work.tile([128, T], bf16, tag="z16", bufs=2)
                    nc.vector.tensor_scalar_add(z16[:, :csz], zp[:, :csz], rb_sb)
                    for g, z_sb in ((0, zi_sb), (1, zj_sb)):
                        gp = pss.tile([PLUP, T], f32, tag="ps")
                        nc.tensor.matmul(
                            gp[:, :csz],
                            lhsT=sel_sb[:, g * PLUP:(g + 1) * PLUP],
                            rhs=z16[:, :csz],
                            start=True,
                            stop=True,
                        )
                        nc.vector.tensor_copy(out=z_sb[:, c0:c0 + csz], in_=gp[:, :csz])

            out_r = out_t.rearrange("(c p) t -> p c t", p=128)

            def p1a(si):
                """DVE: plucker p per tile; p^2 too (kept on DVE)."""
                t0, nt = SEGS[si]
                GL = nt * T
                g0 = HALO + t0 * T
                pp = pp_pool.tile([PLUP, 6, GL], bf16, name=f"pp{si}", tag="pp")
                sq6 = sq_pool.tile([PLUP, 6, GL], bf16, name=f"sq{si}", tag="sq")
                st[si] = {"pp": pp, "sq6": sq6}
                for di, delta in enumerate(OFFSETS):
                    for i in range(nt):
                        past = slice(g0 + i * T - delta, g0 + (i + 1) * T - delta)
                        cur = slice(g0 + i * T, g0 + (i + 1) * T)
                        sl = slice(i * T, (i + 1) * T)
                        m1 = work.tile([PLUP, T], bf16, tag="m1")
                        nc.vector.tensor_mul(m1, zi_sb[:, past], zj_sb[:, cur])
                        m2 = work.tile([PLUP, T], bf16, tag="m2")
                        nc.vector.tensor_mul(m2, zj_sb[:, past], zi_sb[:, cur])
                        nc.vector.tensor_sub(pp[:, di, sl], m1, m2)
                    # p^2 on the (otherwise idle) scalar engine
                    nc.scalar.activation(sq6[:, di, :], pp[:, di, :], AF.Square)

            def norm(si):
                """PE reduce |p|^2 -> Quake rsqrt -> broadcast -> scale."""
                t0, nt = SEGS[si]
                pp, sq6 = st[si]["pp"], st[si]["sq6"]
                NS = 6 * nt
                ns = pss.tile([NSMAX, T], f32, tag="ps", name=f"ns{si}")
                first = True
                for di in range(6):
                    for i in range(nt):
                        nc.tensor.matmul(
                            ns[:NS, :],
                            lhsT=onehot[:, 6 * i + di, :NS],
                            rhs=sq6[:, di, i * T:(i + 1) * T],
                            start=first,
                            stop=(di == 5 and i == nt - 1),
                        )
                        first = False
                # rinv = rsqrt(ns + EPS^2): Quake seed + 1 Newton step
                nsf = rwk.tile([NSMAX, T], f32, tag="rs")
                nc.vector.tensor_scalar_add(nsf[:NS], ns[:NS], 1e-12)
                sh = rwk.tile([NSMAX, T], i32, tag="rs")
                nc.vector.tensor_scalar(
                    sh[:NS], nsf[:NS].bitcast(i32), 1, None,
                    op0=ALU.arith_shift_right,
                )
                y0 = rwk.tile([NSMAX, T], f32, tag="rs")
                nc.vector.tensor_sub(y0[:NS].bitcast(i32), magic[:NS], sh[:NS])
                t1 = rwk.tile([NSMAX, T], f32, tag="rs")
                nc.vector.tensor_mul(t1[:NS], y0[:NS], y0[:NS])
                # t1 = -0.5*y0^2*nsf ; rinv = (t1 + 1.5)*y0
                nc.vector.scalar_tensor_tensor(
                    t1[:NS], t1[:NS], -0.5, nsf[:NS], op0=ALU.mult, op1=ALU.mult,
                )
                rinv = rwk.tile([NSMAX, T], bf16, tag="ri", bufs=2)
                nc.vector.scalar_tensor_tensor(
                    rinv[:NS], t1[:NS], 1.5, y0[:NS], op0=ALU.add, op1=ALU.mult,
                )
                for di in range(6):
                    for i in range(nt):
                        sl = slice(i * T, (i + 1) * T)
                        if BCAST == "gpsimd":
                            rb2 = rbp.tile([128, T], bf16, tag="rb")
                            nc.gpsimd.partition_broadcast(
                                rb2, rinv[6 * i + di:6 * i + di + 1, :]
                            )
                            nc.vector.tensor_mul(
                                pp[:, di, sl], pp[:, di, sl], rb2
                            )
                        else:
                            rb_ps = pss.tile([PLUP, T], f32, tag="ps")
                            nc.tensor.matmul(
                                rb_ps, lhsT=rsel[:NS, 6 * i + di, :],
                                rhs=rinv[:NS, :], start=True, stop=True,
                            )
                            nc.vector.tensor_mul(
                                pp[:, di, sl], pp[:, di, sl], rb_ps
                            )

            def p2(si):
                """a_d = p@g1_w; S = sum_d gelu(a_d + g1_b)."""
                t0, nt = SEGS[si]
                GL = nt * T
                pp = st[si]["pp"]
                s_sb = s_pool.tile([128, 2, GL], bf16, name=f"s{si}", tag="s")
                st[si]["s"] = s_sb
                for di in range(6):
                    for i in range(nt):
                        sl = slice(i * T, (i + 1) * T)
                        for m in range(2):
                            ap_ps = psg.tile([128, T], f32, tag="pg")
                            nc.tensor.matmul(
                                ap_ps,
                                lhsT=g1w_sb[:, m * 128:(m + 1) * 128],
                                rhs=pp[:, di, sl],
                                start=True,
                                stop=True,
                            )
                            if di == 0:
                                nc.scalar.activation(
                                    s_sb[:, m, sl], ap_ps, AF.Gelu,
                                    bias=g1b_sb[:, m:m + 1],
                                )
                            else:
                                gt = work.tile([128, T], bf16, tag="gt")
                                nc.scalar.activation(
                                    gt, ap_ps, AF.Gelu, bias=g1b_sb[:, m:m + 1]
                                )
                                nc.vector.tensor_add(
                                    s_sb[:, m, sl], s_sb[:, m, sl], gt
                                )
                if si == 0:
                    # first-tile count correction (corr==1 for t>=32)
                    for m in range(2):
                        nc.vector.tensor_mul(
                            s_sb[:, m, 0:T], s_sb[:, m, 0:T], corr128
                        )

            def g2part(si):
                """gfm = S @ (g2_w/6) + g2_b (bf16, blend input)."""
                t0, nt = SEGS[si]
                GL = nt * T
                s_sb = st[si]["s"]
                gfm = gfm_pool.tile([128, KD, GL], bf16, name=f"gfm{si}", tag="gfm")
                st[si]["gfm"] = gfm
                for i in range(nt):
                    sl = slice(i * T, (i + 1) * T)
                    for m8 in range(KD):
                        gp = psg.tile([128, T], f32, tag="pg")
                        for k2 in range(2):
                            nc.tensor.matmul(
                                gp,
                                lhsT=g2w_sb[:, k2, m8 * 128:(m8 + 1) * 128],
                                rhs=s_sb[:, k2, sl],
                                start=(k2 == 0),
                                stop=(k2 == 1),
                            )
                        if m8 % 2 == 0:
                            nc.scalar.activation(
                                gfm[:, m8, sl], gp, AF.Identity,
                                bias=g2b_sb[:, m8:m8 + 1],
                            )
                        else:
                            nc.vector.tensor_scalar_add(
                                gfm[:, m8, sl], gp, g2b_sb[:, m8:m8 + 1]
                            )

            def gate(si):
                """logits = h8@(W1*WSC) [fp8 DR] + S@(W2'*WSC) [bf16] in one
                PSUM; alpha = sigmoid(l/WSC + gtb); blend; store."""
                t0, nt = SEGS[si]
                s_sb = st[si]["s"]
                gfm = st[si]["gfm"]
                for i in range(nt):
                    sl = slice(i * T, (i + 1) * T)
                    ti = t0 + i
                    cur = slice(HALO + ti * T, HALO + (ti + 1) * T)
                    for mb in range(2):
                        lps = {}
                        for m8 in range(4 * mb, 4 * mb + 4):
                            lp = psul.tile([128, T], f32, tag="lp")
                            lps[m8] = lp
                            ms = slice(m8 * 128, (m8 + 1) * 128)
                            for kp in range(KD // 2):
                                nc.tensor.matmul(
                                    lp,
                                    lhsT=gw1_sb[:, 2 * kp:2 * kp + 2, ms],
                                    rhs=h8_sb[:, 2 * kp:2 * kp + 2, cur],
                                    start=(kp == 0),
                                    stop=False,
                                    perf_mode=DR,
                                )
                        for m8 in range(4 * mb, 4 * mb + 4):
                            lp = lps[m8]
                            ms = slice(m8 * 128, (m8 + 1) * 128)
                            for k2 in range(2):
                                nc.tensor.matmul(
                                    lp,
                                    lhsT=w2p_sb[:, k2, ms],
                                    rhs=s_sb[:, k2, sl],
                                    start=False,
                                    stop=(k2 == 1),
                                )
                            alpha = work.tile([128, T], bf16, tag="al")
                            nc.scalar.activation(
                                alpha, lp, AF.Sigmoid, bias=gtb_sb[:, m8:m8 + 1],
                                scale=1.0 / WSC,
                            )
                            dd = work.tile([128, T], bf16, tag="dd")
                            nc.vector.tensor_sub(dd, h_sb[:, m8, cur], gfm[:, m8, sl])
                            mm = work.tile([128, T], bf16, tag="mm")
                            nc.vector.tensor_mul(mm, alpha, dd)
                            oo = work.tile([128, T], bf16, tag="oo")
                            nc.vector.tensor_add(oo, gfm[:, m8, sl], mm)
                            nc.sync.dma_start(
                                out=out_r[:, m8, ti * T:(ti + 1) * T], in_=oo
                            )

            # ---- ramped software pipeline ----
            zphase(zchunks[:2])
            p1a(0)
            zphase(zchunks[2:3])
            corr_bcast()
            norm(0)
            p2(0)
            g2part(0)
            zphase(zchunks[3:])
            p1a(1)
            gate(0)
            norm(1)
            p2(1)
            g2part(1)
            gate(1)

    nc.compile()
    return nc


def _get_program():
    if "nc" not in _CACHE:
        _CACHE["nc"] = _build_program()
    return _CACHE["nc"]


def make_in_maps(h, red_w, red_b, g1_w, g1_b, g2_w, g2_b, gate_w, gate_b):
    """Host-side sharding + layout prep. Returns list of 8 input dicts."""
    h = np.asarray(h, np.float32)
    red_w = np.asarray(red_w, np.float32)
    red_b = np.asarray(red_b, np.float32)
    g1_w = np.asarray(g1_w, np.float32)
    g1_b = np.asarray(g1_b, np.float32)
    g2_w = np.asarray(g2_w, np.float32)
    g2_b = np.asarray(g2_b, np.float32)
    gate_w = np.asarray(gate_w, np.float32)
    gate_b = np.asarray(gate_b, np.float32)

    from concourse import mybir as _mb
    F8 = _mb.dt.np(_mb.dt.float8e4)

    W1 = gate_w[:D]
    W2 = gate_w[D:]

    rwp = np.zeros((D, 32), np.float32)
    rwp[:, :R] = red_w
    rw = np.ascontiguousarray(rwp.astype(BF16))
    rb4 = np.zeros((128, 1), np.float32)
    rb4[:R, 0] = red_b
    rb16 = np.ascontiguousarray(rb4)
    selij = np.zeros((128, 2 * PLUP), np.float32)
    for j in range(4):
        for k in range(PLU):
            selij[32 * j + IDX_I[k], k] = 1.0
            selij[32 * j + IDX_J[k], PLUP + k] = 1.0
    selij = np.ascontiguousarray(selij.astype(BF16))
    g1w = np.zeros((PLUP, DG), np.float32)
    g1w[:PLU] = g1_w
    g1w = np.ascontiguousarray(g1w.astype(BF16))
    g1b = np.ascontiguousarray(g1_b.reshape(2, 128).T.astype(np.float32))
    g2wd = np.ascontiguousarray((g2_w / 6.0).astype(BF16))
    g2bd = np.ascontiguousarray(g2_b.reshape(KD, 128).T.astype(np.float32))
    w2p = np.ascontiguousarray((((g2_w / 6.0) @ W2) * WSC).astype(BF16))
    gw1 = np.ascontiguousarray((W1 * WSC).astype(F8))
    gtbd = np.ascontiguousarray(
        (gate_b + g2_b @ W2).reshape(KD, 128).T.astype(np.float32)
    )

    t = np.arange(T)
    count = np.zeros(T, np.float32)
    for d in OFFSETS:
        count += (t >= d)
    corr0 = np.where(count > 0, 6.0 / np.maximum(count, 1.0), 0.0).astype(BF16)
    corr0 = corr0.reshape(1, T)
    corr1 = np.ones((1, T), BF16)

    NSMAX = 6 * max(n for _, n in SEGS)
    rsel = np.zeros((NSMAX, NSMAX, PLUP), np.float32)
    for dd in range(NSMAX):
        rsel[dd, dd, :] = 1.0
    rsel = np.ascontiguousarray(rsel.reshape(NSMAX, NSMAX * PLUP).astype(BF16))

    in_maps = []
    for c in range(NCORES):
        b, half = c // 2, c % 2
        if half == 0:
            pad = np.zeros((HALO, D), np.float32)
        else:
            pad = h[b, half * TOK - HALO: half * TOK]
        hs = np.concatenate([pad, h[b, half * TOK:(half + 1) * TOK]], axis=0)
        h_tt = np.ascontiguousarray(hs.T.astype(BF16))  # (D, TB)
        h8_t = np.ascontiguousarray(hs.T.astype(F8))
        in_maps.append({
            "h_t": h_tt,
            "h8_t": h8_t,
            "rw": rw,
            "rb16": rb16,
            "selij": selij,
            "g1w": g1w,
            "g1b": g1b,
            "g2w": g2wd,
            "g2b": g2bd,
            "w2p": w2p,
            "gw1": gw1,
            "gtb": gtbd,
            "corr": corr0 if half == 0 else corr1,
            "rsel": rsel,
        })
    return in_maps


def assemble_output(results):
    out = np.empty((B, L, D), np.float32)
    for c in range(NCORES):
        b, half = c // 2, c % 2
        ot = np.asarray(results[c]["out_t"]).astype(np.float32)  # (D, TOK)
        out[b, half * TOK:(half + 1) * TOK, :] = ot.T
    return out


def kernel(**inputs):
    from concourse.bass_utils import run_bass_kernel_spmd

    nc = _get_program()
    in_maps = make_in_maps(**inputs)
    res = run_bass_kernel_spmd(nc, in_maps, core_ids=list(range(NCORES)))
    return assemble_output(res.results)
